# revision 33
# baseline (speedup 1.0000x reference)
"""Trainium2 Bass kernel for nn_MixedAttention_16561393893612 (v2).

Computation (reference semantics, fp32 inputs):
  x [B=4, T=2048, D=1024]; first n_s=1984 tokens share QKV weights W_s
  [3D, D]; the last 64 tokens each have their own W_ns[t] [3D, D]; full
  softmax attention (H=16 heads, Dh=64) over all T; out-proj W_out [D, D].

Sharding: tensor-parallel over heads. Core c owns heads (2c, 2c+1):
  QKV projection for its 128-row m-slice, full attention for its 2 heads,
  out-proj partial with the matching 128-column W_out slice; the host
  sums the 8 partial outputs.

v2 redesign vs v1 (which ran a cold half-clock tail phase):
  - ns-token projection uses W_ns as the FWL stationary operand
    ([128n, 128m] tiles; moving xns [128, 4]) so the 50MB weight stream
    enters the PE at 2 elem/lane/cycle instead of 1, and the [m, b]
    outputs patch QT/KT/V directly (no DRAM scratch bounce).
  - W_ns streams chunk-major (all 64 q-slices, then k, then v), so Q_ns
    is ready during batch 0's phase: every attention q-chunk is a full
    512 columns incl. the ns rows -- the separate 64-row ns attention
    pass is gone.
  - k-tiles run in 8 uniform groups of 2; b0/b1 do groups 0..6 during
    their projection phase and spill psO (fp16), with the (14,15) merge
    group emitted inside b2/b3's phases; b2/b3 run all 8 groups in one
    pass (group 7 last, by when ns K/V are patched). No cold tail: the
    kernel ends inside b3's dense attention + out-proj stream.
  - softmax normalization is a reciprocal of the psO ones-row + gpsimd
    partition_broadcast + one DVE multiply per head (replaces the v1
    transpose/stream-shuffle chain).
  - scores issue as back-to-back (h0, h1) pairs at tile_position
    (0,0)/(64,0) so the 64-deep PE window row-tiles them concurrently.
Compute dtype bf16, fp32 PSUM accumulation, fp32 host-side partial sum.
"""

import os
import numpy as np
import ml_dtypes

import bass_rust
import concourse.bass as bass
import concourse.mybir as mybir
import concourse.tile as tile
from concourse.bass_utils import run_bass_kernel_spmd

FP32 = mybir.dt.float32
BF16 = mybir.dt.bfloat16
FP16 = mybir.dt.float16
NPBF16 = ml_dtypes.bfloat16

B, T, D = 4, 2048, 1024
H, DH = 16, 64
NS = 64
N_S = T - NS  # 1984
NCORES = 8
HPC = H // NCORES     # heads per core = 2
M = HPC * DH          # 128
NT = D // 128         # 8 n-tiles (contraction)
KT = T // 128         # 16 k-tiles
QTW = 512             # q tile width
QT = T // QTW         # 4 q tiles
NG = 8                # k-tile groups of 2
SCALE = 1.0 / np.sqrt(DH).astype(np.float32)

# The walrus build on this image rejects instructions carrying more than
# one sync wait ("Too many sync wait commands").  Tile freely emits
# multi-wait instructions, so after tracing we hoist excess waits onto
# injected same-engine NoOps placed immediately before the instruction.
_MAX_WAITS = 1


def _split_waits(nc, max_waits=_MAX_WAITS):
    ctr = 0
    for f in nc.m.functions:
        for blk in f.blocks:
            newlist = []
            for inst in blk.instructions:
                si = inst.sync_info
                waits = list(si.on_wait) if si else []
                if len(waits) > max_waits:
                    head, keep = waits[:-max_waits], waits[-max_waits:]
                    for i in range(0, len(head), max_waits):
                        chunk = head[i : i + max_waits]
                        nop = mybir.InstNoOp(name=f"W-split-{ctr}", ins=[], outs=[])
                        ctr += 1
                        nop.engine = inst.engine
                        nop.sync_info = mybir.SyncInfo(on_wait=chunk, on_update=[])
                        newlist.append(nop)
                    inst.sync_info = mybir.SyncInfo(
                        on_wait=keep, on_update=list(si.on_update)
                    )
                newlist.append(inst)
            blk.instructions[:] = newlist
    return ctr


def _build_program():
    nc = bass.Bass()
    xT_d = nc.dram_tensor("xT", [B, QT, 128, NT, QTW], BF16, kind="ExternalInput")
    wq_d = nc.dram_tensor("wq", [128, NT, M], BF16, kind="ExternalInput")
    wk_d = nc.dram_tensor("wk", [128, NT, M], BF16, kind="ExternalInput")
    wv_d = nc.dram_tensor("wv", [128, NT, M], BF16, kind="ExternalInput")
    wo_d = nc.dram_tensor("wo", [M, D], BF16, kind="ExternalInput")
    # wns[j, tp, p, nt, m] = W_ns[tp, j*D + r0 + m, nt*128 + p]
    wns_d = nc.dram_tensor("wns", [3, NS, 128, NT, 128], BF16, kind="ExternalInput")
    xns_d = nc.dram_tensor("xns", [128, NS, NT, B], BF16, kind="ExternalInput")
    scr_d = nc.dram_tensor("ns_scratch", [3, NS, B, 128], BF16)
    # y stored [b, e-half, t, 512] so each out-proj psum half flushes as
    # one contiguous DMA; the host restores [B, T, D] when summing
    y_d = nc.dram_tensor("y", [B, D // QTW, T, QTW], BF16, kind="ExternalOutput")
    dbg_dump = os.environ.get("DBG_DUMPQKV", "") not in ("", "0")
    if dbg_dump:
        qdbg_d = nc.dram_tensor("qdbg", [M, B, T], BF16, kind="ExternalOutput")
        kdbg_d = nc.dram_tensor("kdbg", [M, B, T], BF16, kind="ExternalOutput")
        vdbg_d = nc.dram_tensor(
            "vdbg", [128, B, KT, HPC, DH + 1], BF16, kind="ExternalOutput"
        )

    from contextlib import ExitStack

    with tile.TileContext(nc) as tc, ExitStack() as ctx:
        sing = ctx.enter_context(tc.tile_pool(name="sing", bufs=1))
        xpool = ctx.enter_context(tc.tile_pool(name="xpool", bufs=5))
        wnspool = ctx.enter_context(tc.tile_pool(name="wnspool", bufs=8))
        ptpool = ctx.enter_context(tc.tile_pool(name="ptpool", bufs=5))
        evac = ctx.enter_context(tc.tile_pool(name="evac", bufs=3))
        otp = ctx.enter_context(tc.tile_pool(name="otp", bufs=10))
        lpool = ctx.enter_context(tc.tile_pool(name="lpool", bufs=6))
        otup = ctx.enter_context(tc.tile_pool(name="otup", bufs=2))
        ps_big = ctx.enter_context(tc.tile_pool(name="ps_big", bufs=2, space="PSUM"))
        ps_pj = ctx.enter_context(tc.tile_pool(name="ps_pj", bufs=2, space="PSUM"))
        ps_o = ctx.enter_context(tc.tile_pool(name="ps_o", bufs=2, space="PSUM"))

        # ---- constants / persistent tensors ----
        wq_sb = sing.tile([128, NT, M], BF16)
        wk_sb = sing.tile([128, NT, M], BF16)
        wv_sb = sing.tile([128, NT, M], BF16)
        wo_sb = sing.tile([M, D], BF16)
        half = NT // 2
        for w_sb_, w_d_ in ((wq_sb, wq_d), (wk_sb, wk_d), (wv_sb, wv_d)):
            nc.sync.dma_start(w_sb_[:, 0:half, :], w_d_[:, 0:half, :])
            nc.sync.dma_start(w_sb_[:, half:NT, :], w_d_[:, half:NT, :])
        nc.sync.dma_start(wo_sb, wo_d[:])

        QT_sb = sing.tile([M, B, T], BF16)          # [m(q rows), b, t]
        KT_sb = sing.tile([M, B, T], BF16)          # [m(k rows), b, t]
        V_sb = sing.tile([128, B, KT, HPC, DH + 1], BF16)  # token-major V
        nc.gpsimd.memset(V_sb[:, :, :, :, DH : DH + 1], 1.0)

        xns_sb = sing.tile([128, NS, NT, B], BF16)
        nc.sync.dma_start(xns_sb, xns_d[:])
        from concourse.masks import make_identity

        ident = sing.tile([128, 128], BF16)
        make_identity(nc, ident)

        # SBUF-resident pre-attn psO spills, all batches (fp16: the l
        # row keeps ~1e-3 precision; range is safe, |l| < 2100)
        spill = {}
        for b_ in range(B):
            for qc_ in range(QT):
                for h_ in range(HPC):
                    spill[(b_, qc_, h_)] = sing.tile(
                        [DH + 1, QTW], FP16, name=f"spl_{b_}_{qc_}_{h_}"
                    )

        # ---- ns projection quanta: one (chunk j, token tp) each ----
        # W_ns slice streams as the MOVING operand (128-row streams keep
        # the HAM clock gate engaged; LDWEIGHTS-heavy forms throttle the
        # whole phase to half clock).  psn = [b 4, m 128] goes through a
        # DRAM bounce (scr) and is patched transposed per (b, j) later.
        _ns_tiles = {}

        def ns_dma(j, tp, dma_eng):
            wt = wnspool.tile([128, NT, 128], BF16, tag="wns", name=f"wns_{j}_{tp}")
            dma_eng.dma_start(wt, wns_d[j, tp])
            _ns_tiles[(j, tp)] = wt

        def ns_mm(j, tp):
            psn = ps_pj.tile([B, 128], FP32, tag="pj", name=f"psn_{j}_{tp}")
            wt = _ns_tiles.pop((j, tp))
            for nt in range(NT):
                nc.tensor.matmul(
                    psn,
                    lhsT=xns_sb[:, tp, nt, :],
                    rhs=wt[:, nt, :],
                    start=(nt == 0),
                    stop=(nt == NT - 1),
                )
            nst = lpool.tile([B, 128], BF16, tag="nst", name=f"nst_{j}_{tp}")
            nc.vector.tensor_copy(nst, psn)
            nc.gpsimd.dma_start(scr_d[j, tp], nst)

        def patch_q(b):
            nsp = lpool.tile([NS, 128], BF16, tag="nsp", name=f"nspq_{b}")
            nc.sync.dma_start(nsp, scr_d[0, :, b, :])
            pst = ps_pj.tile([128, NS], BF16, tag="pj", name=f"pstq_{b}")
            nc.tensor.transpose(pst, nsp, ident[0:NS, 0:NS])
            nc.vector.tensor_copy(QT_sb[:, b, N_S:T], pst)

        def patch_k(b):
            nsp = lpool.tile([NS, 128], BF16, tag="nsp", name=f"nspk_{b}")
            nc.sync.dma_start(nsp, scr_d[1, :, b, :])
            pst = ps_pj.tile([128, NS], BF16, tag="pj", name=f"pstk_{b}")
            nc.tensor.transpose(pst, nsp, ident[0:NS, 0:NS])
            nc.vector.tensor_copy(KT_sb[:, b, N_S:T], pst)

        def patch_v(b):
            # ns V tokens live at kt15 partitions 64..127, token-major
            for h in range(HPC):
                nc.sync.dma_start(
                    V_sb[DH : 2 * DH, b, KT - 1, h, 0:DH],
                    scr_d[2, :, b, h * DH : (h + 1) * DH],
                )

        def v_patch(b):
            # vtmp[:, b, :] is v^T [m 128, tok 64]; PE-transpose it into
            # V_sb kt15 partitions 64..127 (the ns tokens)
            psT = ps_pj.tile([128, 128], BF16, tag="pj", name=f"vpt_{b}")
            nc.tensor.transpose(psT[64:128, :], vtmp[:, b, :], ident)
            nc.vector.tensor_copy(
                V_sb[64:128, b, KT - 1, :, 0:DH],
                psT[64:128, :].rearrange("p (h d) -> p h d", h=HPC),
            )

        # ---- filler queues: independent PE work interleaved between a
        # group's scores and its exp-gated PV so the PE never idles on
        # the Activation engine's exp latency (which otherwise drops the
        # HAM clock gate to half rate).
        ns_fill = []      # ns-projection quanta, consumed first
        op_fill = []      # deferred out-proj chunks, ready to pop
        op_cool = []      # freshly queued chunks; their OT evac (DVE
                          # chain) must finish before the psY matmul, so
                          # they only become poppable at the next qc

        def promote_oproj():
            op_fill.extend(op_cool)
            del op_cool[:]

        def pop_filler(n=1):
            for _ in range(n):
                if ns_fill:
                    ns_fill.pop(0)()
                elif op_fill:
                    op_fill.pop(0)()

        def drain_ns_fill():
            while ns_fill:
                ns_fill.pop(0)()

        # ---- one 2-kt attention group for one q-chunk ----
        # scores issue as (h0, h1) back-to-back pairs (row-tiled PE
        # concurrency); exp per head [128, 1024]; PV accumulates psO.
        def attn_group(b, psO, q0, g, first_kt, last_kt, name, fill=2):
            kts = (2 * g, 2 * g + 1)
            psS = [
                ps_big.tile([128, 2 * QTW], FP32, tag="mm", name=f"psS_{name}_{h}")
                for h in range(HPC)
            ]
            for jj, kt in enumerate(kts):
                for h in range(HPC):
                    nc.tensor.matmul(
                        psS[h][:, jj * QTW : (jj + 1) * QTW],
                        lhsT=KT_sb[
                            h * DH : (h + 1) * DH, b, kt * 128 : (kt + 1) * 128
                        ],
                        rhs=QT_sb[h * DH : (h + 1) * DH, b, q0 : q0 + QTW],
                        start=True,
                        stop=True,
                    )
            pts = []
            for h in range(HPC):
                pt = ptpool.tile([128, 2 * QTW], BF16, tag="pt")
                nc.scalar.activation(
                    pt, psS[h], mybir.ActivationFunctionType.Exp, scale=float(SCALE)
                )
                pts.append(pt)
            pop_filler(fill)
            for h in range(HPC):
                for jj, kt in enumerate(kts):
                    nc.tensor.matmul(
                        psO[h],
                        lhsT=V_sb[:, b, kt, h, :],
                        rhs=pts[h][:, jj * QTW : (jj + 1) * QTW],
                        start=(kt == first_kt),
                        stop=(kt == last_kt),
                    )

        # ---- psO -> OT with softmax normalization ----
        # 1/l from the ones-row (partition 64), gpsimd broadcast to 64
        # partitions, one DVE multiply per head into the stacked OT tile.
        # srcO/srcL: per-head APs for the O rows [64, 512] / l row [1, 512].
        def evac_qc(srcO, srcL, OTt, name):
            # 1/l rows live at partitions 0 and 32 (offsets must be
            # 32-aligned for the BIR verifier); stream_shuffle broadcasts
            # each to the 64 partitions its head's O rows occupy.
            linv2 = lpool.tile([64, QTW], BF16, tag="linv", name=f"li_{name}")
            with nc.allow_low_precision(reason="1/l in bf16"):
                for h in range(HPC):
                    nc.vector.reciprocal(linv2[32 * h : 32 * h + 1, :], srcL[h])
            recb = lpool.tile([DH, HPC, QTW], BF16, tag="recb", name=f"rb_{name}")
            for h in range(HPC):
                nc.vector.stream_shuffle(
                    recb[0:32, h, :], linv2[32 * h : 32 * h + 32, :], [0] * 32
                )
                nc.vector.stream_shuffle(
                    recb[32:DH, h, :], recb[0:32, h, :], [0] * 32
                )
            dbg_unnorm = os.environ.get("DBG_UNNORM", "") not in ("", "0")
            for h in range(HPC):
                if dbg_unnorm:
                    nc.vector.tensor_copy(OTt[h * DH : (h + 1) * DH, :], srcO[h])
                else:
                    nc.vector.tensor_mul(
                        OTt[h * DH : (h + 1) * DH, :], srcO[h], recb[:, h, :]
                    )

        def oproj_chunk(b, OTt, i, tch):
            yt = evac.tile([128, D], BF16, tag="y")
            for e in range(D // QTW):
                psY = ps_pj.tile([128, QTW], FP32, tag="pj")
                nc.tensor.matmul(
                    psY,
                    lhsT=OTt[:, i * 128 : (i + 1) * 128],
                    rhs=wo_sb[:, e * QTW : (e + 1) * QTW],
                    start=True,
                    stop=True,
                )
                # split evacuation between DVE and ACT queues
                if e % 2 == 0:
                    nc.vector.tensor_copy(yt[:, e * QTW : (e + 1) * QTW], psY)
                else:
                    nc.scalar.activation(
                        yt[:, e * QTW : (e + 1) * QTW],
                        psY,
                        mybir.ActivationFunctionType.Copy,
                    )
                nc.gpsimd.dma_start(
                    y_d[b, e, tch * 128 : (tch + 1) * 128, :],
                    yt[:, e * QTW : (e + 1) * QTW],
                )

        def queue_oproj(b, OTt, qc):
            for i in range(4):
                op_cool.append(
                    lambda b_=b, O_=OTt, i_=i, t_=qc * 4 + i: oproj_chunk(
                        b_, O_, i_, t_
                    )
                )

        def drain_fillers():
            drain_ns_fill()
            promote_oproj()
            while op_fill:
                op_fill.pop(0)()

        def proj_batch(b):
            # ---- shared QKV projection for batch b ----
            xts = []
            for qt in range(QT):
                xt = xpool.tile([128, NT, QTW], BF16, tag="xt")
                nc.sync.dma_start(xt, xT_d[b, qt])
                xts.append(xt)
            for qt in range(QT):
                # the last 64 columns of qt3 belong to ns tokens: the
                # shared projection must not clobber the ns patches
                cw = QTW if qt < QT - 1 else QTW - NS
                for w_sb, out_sb in ((wq_sb, QT_sb), (wk_sb, KT_sb)):
                    ps = ps_pj.tile([M, QTW], FP32, tag="pj")
                    for nt in range(NT):
                        nc.tensor.matmul(
                            ps,
                            lhsT=w_sb[:, nt, :],
                            rhs=xts[qt][:, nt, :],
                            start=(nt == 0),
                            stop=(nt == NT - 1),
                        )
                    nc.vector.tensor_copy(
                        out_sb[:, b, qt * QTW : qt * QTW + cw], ps[:, 0:cw]
                    )
                for i in range(QTW // 128):
                    tch = qt * (QTW // 128) + i
                    ps = ps_pj.tile([128, M], FP32, tag="pj")
                    for nt in range(NT):
                        nc.tensor.matmul(
                            ps,
                            lhsT=xts[qt][:, nt, i * 128 : (i + 1) * 128],
                            rhs=wv_sb[:, nt, :],
                            start=(nt == 0),
                            stop=(nt == NT - 1),
                        )
                    rows = 128 if tch < KT - 1 else 64
                    nc.vector.tensor_copy(
                        V_sb[0:rows, b, tch, :, 0:DH],
                        ps[0:rows].rearrange("p (h d) -> p h d", h=HPC),
                    )

        def pre_qc(b, qc, name):
            # groups 0..6 (k-tiles 0..13), then spill psO to SBUF fp16
            q0 = qc * QTW
            psO = [
                ps_o.tile([DH + 1, QTW], FP32, tag="psO", name=f"psOp_{name}_{h}")
                for h in range(HPC)
            ]
            for g in range(NG - 1):
                attn_group(b, psO, q0, g, 0, 13, f"{name}_g{g}", fill=2)
            for h in range(HPC):
                nc.vector.tensor_copy(spill[(b, qc, h)], psO[h])

        def merge_qc(b, qc, name):
            # group 7 (k-tiles 14,15) + spill add + evac + enqueue oproj
            q0 = qc * QTW
            psO = [
                ps_o.tile([DH + 1, QTW], FP32, tag="psO", name=f"psOm_{name}_{h}")
                for h in range(HPC)
            ]
            attn_group(b, psO, q0, NG - 1, 14, 15, f"{name}_g7")
            otu = otup.tile([DH + 1, HPC, QTW], FP32, tag="otu", name=f"otu_{name}")
            for h in range(HPC):
                nc.vector.tensor_add(otu[:, h, :], psO[h], spill[(b, qc, h)])
            OTt = otp.tile([128, QTW], BF16, tag="ot", name=f"ot_{name}")
            evac_qc(
                [otu[0:DH, h, :] for h in range(HPC)],
                [otu[DH : DH + 1, h, :] for h in range(HPC)],
                OTt,
                name,
            )
            queue_oproj(b, OTt, qc)

        def full_qc(b, qc, name):
            # all 8 groups in order (group 7 = k-tiles 14,15 last)
            q0 = qc * QTW
            psO = [
                ps_o.tile([DH + 1, QTW], FP32, tag="psO", name=f"psOf_{name}_{h}")
                for h in range(HPC)
            ]
            for g in range(NG):
                attn_group(b, psO, q0, g, 0, 15, f"{name}_g{g}")
            OTt = otp.tile([128, QTW], BF16, tag="ot", name=f"ot_{name}")
            evac_qc(
                [psO[h][0:DH, :] for h in range(HPC)],
                [psO[h][DH : DH + 1, :] for h in range(HPC)],
                OTt,
                name,
            )
            queue_oproj(b, OTt, qc)

        # ---------------- schedule ----------------
        # ns-projection plan: one (chunk, token) per quantum, q-slices
        # first so Q_ns patches before any batch's qc3 attention; DMA
        # issued 6 quanta (1.5MB) ahead of consumption.
        ns_plan = [(j, tp) for j in range(3) for tp in range(NS)]
        dma_engs = [nc.sync, nc.gpsimd]
        NPF = 6
        for pf in range(NPF):
            ns_dma(*ns_plan[pf], dma_engs[pf % 2])

        def make_ns_quantum(idx):
            def run():
                if idx + NPF < len(ns_plan):
                    ns_dma(*ns_plan[idx + NPF], dma_engs[idx % 2])
                ns_mm(*ns_plan[idx])
            return run

        ns_fill.extend(make_ns_quantum(i) for i in range(len(ns_plan)))

        def drain_ns_to(remaining):
            while len(ns_fill) > remaining:
                pop_filler(1)

        # Phase 0: proj b0, b0 pre qc0-2 (ns chunk-0 quanta as filler)
        p0 = nc.named_scope("proj_b0")
        p0.__enter__()
        proj_batch(0)
        pre_qc(0, 0, "b0q0")
        pre_qc(0, 1, "b0q1")
        pre_qc(0, 2, "b0q2")
        p0.__exit__(None, None, None)

        # Phase 1: proj b1; Q patch; b0 qc3 + b1 pre
        p1 = nc.named_scope("proj_b1")
        p1.__enter__()
        proj_batch(1)
        drain_ns_to(len(ns_plan) - NS)  # chunk 0 done
        for b_ in range(B):
            patch_q(b_)
        pre_qc(0, 3, "b0q3")
        for qc in range(QT):
            pre_qc(1, qc, f"b1q{qc}")
        p1.__exit__(None, None, None)

        # Phase 2: proj b2, b2 pre, then K/V patches + b0 merges
        p2 = nc.named_scope("tail_b2")
        p2.__enter__()
        proj_batch(2)
        for qc in range(QT):
            pre_qc(2, qc, f"b2q{qc}")
        drain_ns_to(0)
        for b_ in range(B):
            patch_k(b_)
            patch_v(b_)
        for qc in range(QT):
            promote_oproj()
            merge_qc(0, qc, f"m0q{qc}")
        p2.__exit__(None, None, None)

        # Phase 3: proj b3, b3 pre interleaved with b1 merges, then
        # b2/b3 merges, drain out-proj
        p3 = nc.named_scope("tail_b3")
        p3.__enter__()
        proj_batch(3)
        for qc in range(QT):
            promote_oproj()
            pre_qc(3, qc, f"b3q{qc}")
            merge_qc(1, qc, f"m1q{qc}")
        for qc in range(QT):
            promote_oproj()
            merge_qc(2, qc, f"m2q{qc}")
            merge_qc(3, qc, f"m3q{qc}")
        drain_fillers()
        p3.__exit__(None, None, None)

        if dbg_dump:
            nc.sync.dma_start(qdbg_d[:], QT_sb)
            nc.sync.dma_start(kdbg_d[:], KT_sb)
            nc.sync.dma_start(vdbg_d[:], V_sb)

    _split_waits(nc)
    return nc


_NC_CACHE = None
LAST_RESULTS = None


def _prep_inputs(x, W_s, W_ns, W_out):
    """Slice/transpose/cast the full inputs into per-core input maps."""
    x = np.asarray(x, dtype=np.float32)
    W_s = np.asarray(W_s, dtype=np.float32)
    W_ns = np.asarray(W_ns, dtype=np.float32)
    W_out = np.asarray(W_out, dtype=np.float32)

    xb = x.astype(NPBF16)
    # xT[b, qt, p, nt, q] = x[b, qt*512+q, nt*128+p]
    xT = np.ascontiguousarray(
        xb.transpose(0, 2, 1)
        .reshape(B, NT, 128, QT, QTW)
        .transpose(0, 3, 2, 1, 4)
    )
    # xns[p, t', nt, b] = x[b, n_s+t', nt*128+p]
    xns = np.ascontiguousarray(
        xb[:, N_S:, :].transpose(2, 1, 0).reshape(NT, 128, NS, B).transpose(1, 2, 0, 3)
    )
    wnsb = W_ns.astype(NPBF16)
    wsb = W_s.astype(NPBF16)
    wob = W_out.astype(NPBF16)

    in_maps = []
    for c in range(NCORES):
        r0 = c * M

        def wslice(rows):
            w = wsb[rows, :]  # [M, D]
            return np.ascontiguousarray(
                w.T.reshape(NT, 128, M).transpose(1, 0, 2)
            )

        wq = wslice(slice(r0, r0 + M))
        wk = wslice(slice(D + r0, D + r0 + M))
        wv = wslice(slice(2 * D + r0, 2 * D + r0 + M))
        wo = np.ascontiguousarray(wob[:, c * M : (c + 1) * M].T)
        # wns[j, tp, p, nt, m] = W_ns[tp, j*D + r0 + m, nt*128 + p]
        wns = np.empty((3, NS, 128, NT, 128), dtype=NPBF16)
        for j in range(3):
            sl = wnsb[:, j * D + r0 : j * D + r0 + M, :]  # [NS, 128m, 1024n]
            wns[j] = (
                sl.transpose(0, 2, 1)          # [NS, n, m]
                .reshape(NS, NT, 128, M)       # [NS, nt, p, m]
                .transpose(0, 2, 1, 3)         # [NS, p, nt, m]
            )
        wns = np.ascontiguousarray(wns)
        in_maps.append(
            {"xT": xT, "wq": wq, "wk": wk, "wv": wv, "wo": wo, "wns": wns, "xns": xns}
        )
    return in_maps


def kernel(x, n_s, W_s, W_ns, W_out):
    global _NC_CACHE, LAST_RESULTS
    assert int(n_s) == N_S, f"kernel compiled for n_s={N_S}, got {int(n_s)}"
    in_maps = _prep_inputs(x, W_s, W_ns, W_out)
    if _NC_CACHE is None:
        _NC_CACHE = _build_program()
    nc = _NC_CACHE
    trace = os.environ.get("BASS_TRACE", "") not in ("", "0")
    kwargs = {}
    if trace:
        stitch = os.environ.get("BASS_STITCH", "") not in ("", "0")
        kwargs = dict(
            trace=True, trace_cores=list(range(NCORES)), stitch_traces=stitch
        )
    res = run_bass_kernel_spmd(nc, in_maps, core_ids=list(range(NCORES)), **kwargs)
    LAST_RESULTS = res
    out = np.zeros((B, T, D), dtype=np.float32)
    for c in range(NCORES):
        yc = res.results[c]["y"]  # [B, D//QTW, T, QTW]
        out += yc.transpose(0, 2, 1, 3).reshape(B, T, D).astype(np.float32)
    return out


# revision 39
# speedup vs baseline: 1.0482x; 1.0482x over previous
"""Trainium2 Bass kernel for nn_MixedAttention_16561393893612 (v2).

Computation (reference semantics, fp32 inputs):
  x [B=4, T=2048, D=1024]; first n_s=1984 tokens share QKV weights W_s
  [3D, D]; the last 64 tokens each have their own W_ns[t] [3D, D]; full
  softmax attention (H=16 heads, Dh=64) over all T; out-proj W_out [D, D].

Sharding: tensor-parallel over heads. Core c owns heads (2c, 2c+1):
  QKV projection for its 128-row m-slice, full attention for its 2 heads,
  out-proj partial with the matching 128-column W_out slice; the host
  sums the 8 partial outputs.

v2 redesign vs v1 (which ran a cold half-clock tail phase):
  - ns-token projection uses W_ns as the FWL stationary operand
    ([128n, 128m] tiles; moving xns [128, 4]) so the 50MB weight stream
    enters the PE at 2 elem/lane/cycle instead of 1, and the [m, b]
    outputs patch QT/KT/V directly (no DRAM scratch bounce).
  - W_ns streams chunk-major (all 64 q-slices, then k, then v), so Q_ns
    is ready during batch 0's phase: every attention q-chunk is a full
    512 columns incl. the ns rows -- the separate 64-row ns attention
    pass is gone.
  - k-tiles run in 8 uniform groups of 2; b0/b1 do groups 0..6 during
    their projection phase and spill psO (fp16), with the (14,15) merge
    group emitted inside b2/b3's phases; b2/b3 run all 8 groups in one
    pass (group 7 last, by when ns K/V are patched). No cold tail: the
    kernel ends inside b3's dense attention + out-proj stream.
  - softmax normalization is a reciprocal of the psO ones-row + gpsimd
    partition_broadcast + one DVE multiply per head (replaces the v1
    transpose/stream-shuffle chain).
  - scores issue as back-to-back (h0, h1) pairs at tile_position
    (0,0)/(64,0) so the 64-deep PE window row-tiles them concurrently.
Compute dtype bf16, fp32 PSUM accumulation, fp32 host-side partial sum.
"""

import os
import numpy as np
import ml_dtypes

import bass_rust
import concourse.bass as bass
import concourse.mybir as mybir
import concourse.tile as tile
from concourse.bass_utils import run_bass_kernel_spmd

FP32 = mybir.dt.float32
BF16 = mybir.dt.bfloat16
FP16 = mybir.dt.float16
NPBF16 = ml_dtypes.bfloat16

B, T, D = 4, 2048, 1024
H, DH = 16, 64
NS = 64
N_S = T - NS  # 1984
NCORES = 8
HPC = H // NCORES     # heads per core = 2
M = HPC * DH          # 128
NT = D // 128         # 8 n-tiles (contraction)
KT = T // 128         # 16 k-tiles
QTW = 512             # q tile width
QT = T // QTW         # 4 q tiles
NG = 8                # k-tile groups of 2
SCALE = 1.0 / np.sqrt(DH).astype(np.float32)

# The walrus build on this image rejects instructions carrying more than
# one sync wait ("Too many sync wait commands").  Tile freely emits
# multi-wait instructions, so after tracing we hoist excess waits onto
# injected same-engine NoOps placed immediately before the instruction.
_MAX_WAITS = 1


def _split_waits(nc, max_waits=_MAX_WAITS):
    ctr = 0
    for f in nc.m.functions:
        for blk in f.blocks:
            newlist = []
            for inst in blk.instructions:
                si = inst.sync_info
                waits = list(si.on_wait) if si else []
                if len(waits) > max_waits:
                    head, keep = waits[:-max_waits], waits[-max_waits:]
                    for i in range(0, len(head), max_waits):
                        chunk = head[i : i + max_waits]
                        nop = mybir.InstNoOp(name=f"W-split-{ctr}", ins=[], outs=[])
                        ctr += 1
                        nop.engine = inst.engine
                        nop.sync_info = mybir.SyncInfo(on_wait=chunk, on_update=[])
                        newlist.append(nop)
                    inst.sync_info = mybir.SyncInfo(
                        on_wait=keep, on_update=list(si.on_update)
                    )
                newlist.append(inst)
            blk.instructions[:] = newlist
    return ctr


def _build_program():
    nc = bass.Bass()
    xT_d = nc.dram_tensor("xT", [B, QT, 128, NT, QTW], BF16, kind="ExternalInput")
    wq_d = nc.dram_tensor("wq", [128, NT, M], BF16, kind="ExternalInput")
    wk_d = nc.dram_tensor("wk", [128, NT, M], BF16, kind="ExternalInput")
    wv_d = nc.dram_tensor("wv", [128, NT, M], BF16, kind="ExternalInput")
    wo_d = nc.dram_tensor("wo", [M, D], BF16, kind="ExternalInput")
    # wns[j, tp, p, nt, m] = W_ns[tp, j*D + r0 + m, nt*128 + p]
    wns_d = nc.dram_tensor("wns", [3, NS, 128, NT, 128], BF16, kind="ExternalInput")
    xns_d = nc.dram_tensor("xns", [128, NS, NT, B], BF16, kind="ExternalInput")
    scr_d = nc.dram_tensor("ns_scratch", [3, NS, B, 128], BF16)
    # y stored [b, e-half, t, 512] so each out-proj psum half flushes as
    # one contiguous DMA; the host restores [B, T, D] when summing
    y_d = nc.dram_tensor("y", [B, D // QTW, T, QTW], BF16, kind="ExternalOutput")
    dbg_dump = os.environ.get("DBG_DUMPQKV", "") not in ("", "0")
    if dbg_dump:
        qdbg_d = nc.dram_tensor("qdbg", [M, B, T], BF16, kind="ExternalOutput")
        kdbg_d = nc.dram_tensor("kdbg", [M, B, T], BF16, kind="ExternalOutput")
        vdbg_d = nc.dram_tensor(
            "vdbg", [128, B, KT, HPC, DH + 1], BF16, kind="ExternalOutput"
        )

    from contextlib import ExitStack

    with tile.TileContext(nc) as tc, ExitStack() as ctx:
        sing = ctx.enter_context(tc.tile_pool(name="sing", bufs=1))
        xpool = ctx.enter_context(tc.tile_pool(name="xpool", bufs=5))
        wnspool = ctx.enter_context(tc.tile_pool(name="wnspool", bufs=8))
        ptpool = ctx.enter_context(tc.tile_pool(name="ptpool", bufs=5))
        evac = ctx.enter_context(tc.tile_pool(name="evac", bufs=4))
        otp = ctx.enter_context(tc.tile_pool(name="otp", bufs=10))
        lpool = ctx.enter_context(tc.tile_pool(name="lpool", bufs=6))
        ps_big = ctx.enter_context(tc.tile_pool(name="ps_big", bufs=2, space="PSUM"))
        ps_pj = ctx.enter_context(tc.tile_pool(name="ps_pj", bufs=2, space="PSUM"))
        ps_o = ctx.enter_context(tc.tile_pool(name="ps_o", bufs=2, space="PSUM"))

        # ---- constants / persistent tensors ----
        wq_sb = sing.tile([128, NT, M], BF16)
        wk_sb = sing.tile([128, NT, M], BF16)
        wv_sb = sing.tile([128, NT, M], BF16)
        wo_sb = sing.tile([M, D], BF16)
        half = NT // 2
        for w_sb_, w_d_ in ((wq_sb, wq_d), (wk_sb, wk_d), (wv_sb, wv_d)):
            nc.sync.dma_start(w_sb_[:, 0:half, :], w_d_[:, 0:half, :])
            nc.sync.dma_start(w_sb_[:, half:NT, :], w_d_[:, half:NT, :])
        nc.sync.dma_start(wo_sb, wo_d[:])

        QT_sb = sing.tile([M, B, T], BF16)          # [m(q rows), b, t]
        KT_sb = sing.tile([M, B, T], BF16)          # [m(k rows), b, t]
        V_sb = sing.tile([128, B, KT, HPC, DH + 1], BF16)  # token-major V
        nc.gpsimd.memset(V_sb[:, :, :, :, DH : DH + 1], 1.0)

        xns_sb = sing.tile([128, NS, NT, B], BF16)
        nc.sync.dma_start(xns_sb, xns_d[:])
        from concourse.masks import make_identity

        ident = sing.tile([128, 128], BF16)
        make_identity(nc, ident)

        # ones rows (partitions 0 and 32) for the PE 1/l broadcast
        ones_sb = sing.tile([33, 128], BF16)
        nc.gpsimd.memset(ones_sb[0:1, :], 1.0)
        nc.gpsimd.memset(ones_sb[32:33, :], 1.0)

        # SBUF-resident pre-attn psO spills, all batches (bf16; merged
        # back into psum via an identity matmul, not a DVE add)
        spill = {}
        for b_ in range(B):
            for qc_ in range(QT):
                for h_ in range(HPC):
                    spill[(b_, qc_, h_)] = sing.tile(
                        [DH + 1, QTW], BF16, name=f"spl_{b_}_{qc_}_{h_}"
                    )

        # ---- ns projection quanta: one (chunk j, token tp) each ----
        # W_ns slice streams as the MOVING operand (128-row streams keep
        # the HAM clock gate engaged; LDWEIGHTS-heavy forms throttle the
        # whole phase to half clock).  psn = [b 4, m 128] goes through a
        # DRAM bounce (scr) and is patched transposed per (b, j) later.
        _ns_tiles = {}

        def ns_dma(j, tp, dma_eng):
            wt = wnspool.tile([128, NT, 128], BF16, tag="wns", name=f"wns_{j}_{tp}")
            dma_eng.dma_start(wt, wns_d[j, tp])
            _ns_tiles[(j, tp)] = wt

        def ns_mm(j, tp):
            psn = ps_pj.tile([B, 128], FP32, tag="pj", name=f"psn_{j}_{tp}")
            wt = _ns_tiles.pop((j, tp))
            for nt in range(NT):
                nc.tensor.matmul(
                    psn,
                    lhsT=xns_sb[:, tp, nt, :],
                    rhs=wt[:, nt, :],
                    start=(nt == 0),
                    stop=(nt == NT - 1),
                )
            nst = lpool.tile([B, 128], BF16, tag="nst", name=f"nst_{j}_{tp}")
            nc.vector.tensor_copy(nst, psn)
            nc.gpsimd.dma_start(scr_d[j, tp], nst)

        def patch_q(b):
            nsp = lpool.tile([NS, 128], BF16, tag="nsp", name=f"nspq_{b}")
            nc.sync.dma_start(nsp, scr_d[0, :, b, :])
            pst = ps_pj.tile([128, NS], BF16, tag="pj", name=f"pstq_{b}")
            nc.tensor.transpose(pst, nsp, ident[0:NS, 0:NS])
            nc.vector.tensor_copy(QT_sb[:, b, N_S:T], pst)

        def patch_k(b):
            nsp = lpool.tile([NS, 128], BF16, tag="nsp", name=f"nspk_{b}")
            nc.sync.dma_start(nsp, scr_d[1, :, b, :])
            pst = ps_pj.tile([128, NS], BF16, tag="pj", name=f"pstk_{b}")
            nc.tensor.transpose(pst, nsp, ident[0:NS, 0:NS])
            nc.vector.tensor_copy(KT_sb[:, b, N_S:T], pst)

        def patch_v(b):
            # ns V tokens live at kt15 partitions 64..127, token-major
            for h in range(HPC):
                nc.sync.dma_start(
                    V_sb[DH : 2 * DH, b, KT - 1, h, 0:DH],
                    scr_d[2, :, b, h * DH : (h + 1) * DH],
                )

        def v_patch(b):
            # vtmp[:, b, :] is v^T [m 128, tok 64]; PE-transpose it into
            # V_sb kt15 partitions 64..127 (the ns tokens)
            psT = ps_pj.tile([128, 128], BF16, tag="pj", name=f"vpt_{b}")
            nc.tensor.transpose(psT[64:128, :], vtmp[:, b, :], ident)
            nc.vector.tensor_copy(
                V_sb[64:128, b, KT - 1, :, 0:DH],
                psT[64:128, :].rearrange("p (h d) -> p h d", h=HPC),
            )

        # ---- filler queues: independent PE work interleaved between a
        # group's scores and its exp-gated PV so the PE never idles on
        # the Activation engine's exp latency (which otherwise drops the
        # HAM clock gate to half rate).
        ns_fill = []      # ns-projection quanta, consumed first
        op_fill = []      # deferred out-proj chunks, ready to pop
        op_cool = []      # freshly queued chunks; their OT evac (DVE
                          # chain) must finish before the psY matmul, so
                          # they only become poppable at the next qc

        def promote_oproj():
            op_fill.extend(op_cool)
            del op_cool[:]

        def pop_filler(n=1):
            for _ in range(n):
                if ns_fill:
                    ns_fill.pop(0)()
                elif op_fill:
                    op_fill.pop(0)()

        def drain_ns_fill():
            while ns_fill:
                ns_fill.pop(0)()

        # ---- one 2-kt attention group for one q-chunk ----
        # scores issue as (h0, h1) back-to-back pairs (row-tiled PE
        # concurrency); exp per head [128, 1024]; PV accumulates psO.
        def attn_group(b, psO, q0, g, first_kt, last_kt, name, fill=2):
            kts = (2 * g, 2 * g + 1)
            psS = [
                ps_big.tile([128, 2 * QTW], FP32, tag="mm", name=f"psS_{name}_{h}")
                for h in range(HPC)
            ]
            for jj, kt in enumerate(kts):
                for h in range(HPC):
                    nc.tensor.matmul(
                        psS[h][:, jj * QTW : (jj + 1) * QTW],
                        lhsT=KT_sb[
                            h * DH : (h + 1) * DH, b, kt * 128 : (kt + 1) * 128
                        ],
                        rhs=QT_sb[h * DH : (h + 1) * DH, b, q0 : q0 + QTW],
                        start=True,
                        stop=True,
                    )
            pts = []
            for h in range(HPC):
                pt = ptpool.tile([128, 2 * QTW], BF16, tag="pt")
                nc.scalar.activation(
                    pt, psS[h], mybir.ActivationFunctionType.Exp, scale=float(SCALE)
                )
                pts.append(pt)
            pop_filler(fill)
            for h in range(HPC):
                for jj, kt in enumerate(kts):
                    nc.tensor.matmul(
                        psO[h],
                        lhsT=V_sb[:, b, kt, h, :],
                        rhs=pts[h][:, jj * QTW : (jj + 1) * QTW],
                        start=(kt == first_kt),
                        stop=(kt == last_kt),
                    )

        # ---- psO -> OT with softmax normalization ----
        # 1/l: ACT reciprocal of the ones-row into partitions 0/32, PE
        # outer-product broadcast into a psum tile, one DVE multiply per
        # head into the stacked OT tile.
        # srcO/srcL: per-head APs for the O rows [64, 512] / l row [1, 512].
        def evac_qc(srcO, srcL, OTt, name):
            linv2 = lpool.tile([33, QTW], BF16, tag="linv", name=f"li_{name}")
            with nc.allow_low_precision(reason="1/l in bf16"):
                for h in range(HPC):
                    nc.vector.reciprocal(linv2[32 * h : 32 * h + 1, :], srcL[h])
            psB = ps_pj.tile([128, QTW], FP32, tag="pj", name=f"pb_{name}")
            for h in range(HPC):
                nc.tensor.matmul(
                    psB[h * DH : (h + 1) * DH, :],
                    lhsT=ones_sb[32 * h : 32 * h + 1, 0:DH],
                    rhs=linv2[32 * h : 32 * h + 1, :],
                    start=True,
                    stop=True,
                )
            # tensor_tensor reads at most one PSUM operand: stage the
            # broadcast in SBUF via the ACT queue
            recb = lpool.tile([128, QTW], BF16, tag="recb", name=f"rb_{name}")
            nc.scalar.activation(recb, psB, mybir.ActivationFunctionType.Copy)
            for h in range(HPC):
                nc.vector.tensor_mul(
                    OTt[h * DH : (h + 1) * DH, :],
                    srcO[h],
                    recb[h * DH : (h + 1) * DH, :],
                )

        def oproj_chunk(b, OTt, i, tch):
            yt = evac.tile([128, D], BF16, tag="y")
            for e in range(D // QTW):
                psY = ps_pj.tile([128, QTW], FP32, tag="pj")
                nc.tensor.matmul(
                    psY,
                    lhsT=OTt[:, i * 128 : (i + 1) * 128],
                    rhs=wo_sb[:, e * QTW : (e + 1) * QTW],
                    start=True,
                    stop=True,
                )
                # split evacuation between DVE and ACT queues
                if e % 2 == 0:
                    nc.vector.tensor_copy(yt[:, e * QTW : (e + 1) * QTW], psY)
                else:
                    nc.scalar.activation(
                        yt[:, e * QTW : (e + 1) * QTW],
                        psY,
                        mybir.ActivationFunctionType.Copy,
                    )
                nc.gpsimd.dma_start(
                    y_d[b, e, tch * 128 : (tch + 1) * 128, :],
                    yt[:, e * QTW : (e + 1) * QTW],
                )

        def queue_oproj(b, OTt, qc):
            for i in range(4):
                op_cool.append(
                    lambda b_=b, O_=OTt, i_=i, t_=qc * 4 + i: oproj_chunk(
                        b_, O_, i_, t_
                    )
                )

        def drain_fillers():
            drain_ns_fill()
            promote_oproj()
            while op_fill:
                op_fill.pop(0)()

        def proj_batch(b):
            # ---- shared QKV projection for batch b ----
            xts = []
            for qt in range(QT):
                xt = xpool.tile([128, NT, QTW], BF16, tag="xt")
                nc.sync.dma_start(xt, xT_d[b, qt])
                xts.append(xt)
            for qt in range(QT):
                # the last 64 columns of qt3 belong to ns tokens: the
                # shared projection must not clobber the ns patches
                cw = QTW if qt < QT - 1 else QTW - NS
                for w_sb, out_sb in ((wq_sb, QT_sb), (wk_sb, KT_sb)):
                    ps = ps_pj.tile([M, QTW], FP32, tag="pj")
                    for nt in range(NT):
                        nc.tensor.matmul(
                            ps,
                            lhsT=w_sb[:, nt, :],
                            rhs=xts[qt][:, nt, :],
                            start=(nt == 0),
                            stop=(nt == NT - 1),
                        )
                    nc.vector.tensor_copy(
                        out_sb[:, b, qt * QTW : qt * QTW + cw], ps[:, 0:cw]
                    )
                for i in range(QTW // 128):
                    tch = qt * (QTW // 128) + i
                    ps = ps_pj.tile([128, M], FP32, tag="pj")
                    for nt in range(NT):
                        nc.tensor.matmul(
                            ps,
                            lhsT=xts[qt][:, nt, i * 128 : (i + 1) * 128],
                            rhs=wv_sb[:, nt, :],
                            start=(nt == 0),
                            stop=(nt == NT - 1),
                        )
                    rows = 128 if tch < KT - 1 else 64
                    nc.vector.tensor_copy(
                        V_sb[0:rows, b, tch, :, 0:DH],
                        ps[0:rows].rearrange("p (h d) -> p h d", h=HPC),
                    )

        def pre_qc(b, qc, name):
            # groups 0..6 (k-tiles 0..13), then spill psO to SBUF fp16
            q0 = qc * QTW
            psO = [
                ps_o.tile([DH + 1, QTW], FP32, tag="psO", name=f"psOp_{name}_{h}")
                for h in range(HPC)
            ]
            for g in range(NG - 1):
                attn_group(b, psO, q0, g, 0, 13, f"{name}_g{g}", fill=2)
            for h in range(HPC):
                nc.vector.tensor_copy(spill[(b, qc, h)], psO[h])

        def merge_qc(b, qc, name):
            # group 7 (k-tiles 14,15); the pre-phase spill re-enters the
            # accumulator via an identity matmul (start=True), so PV just
            # accumulates on top and the evac reads psO directly.
            q0 = qc * QTW
            psO = [
                ps_o.tile([DH + 1, QTW], FP32, tag="psO", name=f"psOm_{name}_{h}")
                for h in range(HPC)
            ]
            for h in range(HPC):
                nc.tensor.matmul(
                    psO[h],
                    lhsT=ident[0 : DH + 1, 0 : DH + 1],
                    rhs=spill[(b, qc, h)],
                    start=True,
                    stop=False,
                )
            attn_group(b, psO, q0, NG - 1, -1, 15, f"{name}_g7")
            OTt = otp.tile([128, QTW], BF16, tag="ot", name=f"ot_{name}")
            evac_qc(
                [psO[h][0:DH, :] for h in range(HPC)],
                [psO[h][DH : DH + 1, :] for h in range(HPC)],
                OTt,
                name,
            )
            queue_oproj(b, OTt, qc)

        def full_qc(b, qc, name):
            # all 8 groups in order (group 7 = k-tiles 14,15 last)
            q0 = qc * QTW
            psO = [
                ps_o.tile([DH + 1, QTW], FP32, tag="psO", name=f"psOf_{name}_{h}")
                for h in range(HPC)
            ]
            for g in range(NG):
                attn_group(b, psO, q0, g, 0, 15, f"{name}_g{g}")
            OTt = otp.tile([128, QTW], BF16, tag="ot", name=f"ot_{name}")
            evac_qc(
                [psO[h][0:DH, :] for h in range(HPC)],
                [psO[h][DH : DH + 1, :] for h in range(HPC)],
                OTt,
                name,
            )
            queue_oproj(b, OTt, qc)

        # ---------------- schedule ----------------
        # ns-projection plan: one (chunk, token) per quantum, q-slices
        # first so Q_ns patches before any batch's qc3 attention; DMA
        # issued 6 quanta (1.5MB) ahead of consumption.
        ns_plan = [(j, tp) for j in range(3) for tp in range(NS)]
        dma_engs = [nc.sync, nc.gpsimd]
        NPF = 6
        for pf in range(NPF):
            ns_dma(*ns_plan[pf], dma_engs[pf % 2])

        def make_ns_quantum(idx):
            def run():
                if idx + NPF < len(ns_plan):
                    ns_dma(*ns_plan[idx + NPF], dma_engs[idx % 2])
                ns_mm(*ns_plan[idx])
            return run

        ns_fill.extend(make_ns_quantum(i) for i in range(len(ns_plan)))

        def drain_ns_to(remaining):
            while len(ns_fill) > remaining:
                pop_filler(1)

        # Phase 0: proj b0, b0 pre qc0-2 (ns chunk-0 quanta as filler)
        p0 = nc.named_scope("proj_b0")
        p0.__enter__()
        proj_batch(0)
        pre_qc(0, 0, "b0q0")
        pre_qc(0, 1, "b0q1")
        pre_qc(0, 2, "b0q2")
        p0.__exit__(None, None, None)

        # Phase 1: proj b1; Q patch; b0 qc3 + b1 pre
        p1 = nc.named_scope("proj_b1")
        p1.__enter__()
        proj_batch(1)
        drain_ns_to(len(ns_plan) - NS)  # chunk 0 done
        for b_ in range(B):
            patch_q(b_)
        pre_qc(0, 3, "b0q3")
        for qc in range(QT):
            pre_qc(1, qc, f"b1q{qc}")
        p1.__exit__(None, None, None)

        # Phase 2: proj b2, b2 pre, then K/V patches + b0 merges
        p2 = nc.named_scope("tail_b2")
        p2.__enter__()
        proj_batch(2)
        for qc in range(QT):
            pre_qc(2, qc, f"b2q{qc}")
        drain_ns_to(0)
        for b_ in range(B):
            patch_k(b_)
            patch_v(b_)
        for qc in range(QT):
            promote_oproj()
            merge_qc(0, qc, f"m0q{qc}")
        p2.__exit__(None, None, None)

        # Phase 3: proj b3, b3 pre interleaved with b1 merges, then
        # b2/b3 merges, drain out-proj
        p3 = nc.named_scope("tail_b3")
        p3.__enter__()
        proj_batch(3)
        for qc in range(QT):
            promote_oproj()
            pre_qc(3, qc, f"b3q{qc}")
            merge_qc(1, qc, f"m1q{qc}")
        for qc in range(QT):
            promote_oproj()
            merge_qc(2, qc, f"m2q{qc}")
            merge_qc(3, qc, f"m3q{qc}")
        drain_fillers()
        p3.__exit__(None, None, None)

        if dbg_dump:
            nc.sync.dma_start(qdbg_d[:], QT_sb)
            nc.sync.dma_start(kdbg_d[:], KT_sb)
            nc.sync.dma_start(vdbg_d[:], V_sb)

    _split_waits(nc)
    return nc


_NC_CACHE = None
LAST_RESULTS = None


def _prep_inputs(x, W_s, W_ns, W_out):
    """Slice/transpose/cast the full inputs into per-core input maps."""
    x = np.asarray(x, dtype=np.float32)
    W_s = np.asarray(W_s, dtype=np.float32)
    W_ns = np.asarray(W_ns, dtype=np.float32)
    W_out = np.asarray(W_out, dtype=np.float32)

    xb = x.astype(NPBF16)
    # xT[b, qt, p, nt, q] = x[b, qt*512+q, nt*128+p]
    xT = np.ascontiguousarray(
        xb.transpose(0, 2, 1)
        .reshape(B, NT, 128, QT, QTW)
        .transpose(0, 3, 2, 1, 4)
    )
    # xns[p, t', nt, b] = x[b, n_s+t', nt*128+p]
    xns = np.ascontiguousarray(
        xb[:, N_S:, :].transpose(2, 1, 0).reshape(NT, 128, NS, B).transpose(1, 2, 0, 3)
    )
    wnsb = W_ns.astype(NPBF16)
    wsb = W_s.astype(NPBF16)
    wob = W_out.astype(NPBF16)

    in_maps = []
    for c in range(NCORES):
        r0 = c * M

        def wslice(rows):
            w = wsb[rows, :]  # [M, D]
            return np.ascontiguousarray(
                w.T.reshape(NT, 128, M).transpose(1, 0, 2)
            )

        wq = wslice(slice(r0, r0 + M))
        wk = wslice(slice(D + r0, D + r0 + M))
        wv = wslice(slice(2 * D + r0, 2 * D + r0 + M))
        wo = np.ascontiguousarray(wob[:, c * M : (c + 1) * M].T)
        # wns[j, tp, p, nt, m] = W_ns[tp, j*D + r0 + m, nt*128 + p]
        wns = np.empty((3, NS, 128, NT, 128), dtype=NPBF16)
        for j in range(3):
            sl = wnsb[:, j * D + r0 : j * D + r0 + M, :]  # [NS, 128m, 1024n]
            wns[j] = (
                sl.transpose(0, 2, 1)          # [NS, n, m]
                .reshape(NS, NT, 128, M)       # [NS, nt, p, m]
                .transpose(0, 2, 1, 3)         # [NS, p, nt, m]
            )
        wns = np.ascontiguousarray(wns)
        in_maps.append(
            {"xT": xT, "wq": wq, "wk": wk, "wv": wv, "wo": wo, "wns": wns, "xns": xns}
        )
    return in_maps


def kernel(x, n_s, W_s, W_ns, W_out):
    global _NC_CACHE, LAST_RESULTS
    assert int(n_s) == N_S, f"kernel compiled for n_s={N_S}, got {int(n_s)}"
    in_maps = _prep_inputs(x, W_s, W_ns, W_out)
    if _NC_CACHE is None:
        _NC_CACHE = _build_program()
    nc = _NC_CACHE
    trace = os.environ.get("BASS_TRACE", "") not in ("", "0")
    kwargs = {}
    if trace:
        stitch = os.environ.get("BASS_STITCH", "") not in ("", "0")
        kwargs = dict(
            trace=True, trace_cores=list(range(NCORES)), stitch_traces=stitch
        )
    res = run_bass_kernel_spmd(nc, in_maps, core_ids=list(range(NCORES)), **kwargs)
    LAST_RESULTS = res
    out = np.zeros((B, T, D), dtype=np.float32)
    for c in range(NCORES):
        yc = res.results[c]["y"]  # [B, D//QTW, T, QTW]
        out += yc.transpose(0, 2, 1, 3).reshape(B, T, D).astype(np.float32)
    return out
